# revision 55
# baseline (speedup 1.0000x reference)
import os
import sys

sys.path.insert(0, "/opt/trn_rl_repo")

import numpy as np
import ml_dtypes

import concourse.bass as bass
import concourse.bacc as bacc
import concourse.tile as tile
from concourse import mybir
from concourse.bass import ds, ts

BF16 = ml_dtypes.bfloat16

B, N, C = 2, 2048, 1024
H = 16
HD = C // H          # 64
HPC = 4              # heads per core
NCORES = 8
SCALE = HD ** -0.5   # 0.125
KT = C // 128        # 8 k-tiles over the C contraction
KTA = KT + 1         # +1 aug tile (bias / ones row)
FQ = HPC * HD        # 256 per-core q/k feature dim
VEXT = HPC * (HD + 1)  # 260: per-head [64 v cols | 1 ones col]
NT = N // 128        # 16 row tiles
NQC = N // 512       # 4 query chunks of 512

# Schraudolph bf16 exp: bits(exp(s)) ~= round(A_SCH*s + B_SCH), read as bf16.
# q,k weights are pre-scaled by sqrt(SCALE*A_SCH) so the score PSUM arrives
# already in the Schraudolph domain; the exact-exp path divides back out.
A_SCH = 184.6650292  # 2^7 / ln 2
C_SCH = 7.5          # bias correction (tuned for softmax-normalized output)
B_SCH = 16256.0 - C_SCH
S_QK = float(np.sqrt(SCALE * A_SCH))


def _build_nc(hw_passes: bool = True) -> bass.Bass:
    nc = bass.Bass()
    f32 = mybir.dt.float32
    bf16 = mybir.dt.bfloat16
    u16 = mybir.dt.uint16

    xt_d = nc.dram_tensor("xt", [KT, 128, N], bf16, kind="ExternalInput")
    wq_d = nc.dram_tensor("wq", [KTA, 128, FQ], bf16, kind="ExternalInput")
    wk_d = nc.dram_tensor("wk", [KTA, 128, FQ], bf16, kind="ExternalInput")
    wv_d = nc.dram_tensor("wv", [KTA, 128, VEXT], bf16, kind="ExternalInput")
    wp_d = nc.dram_tensor("wp", [2, 128, C], bf16, kind="ExternalInput")
    ind_d = nc.dram_tensor("ind", [HPC, HPC, HD], bf16, kind="ExternalInput")
    out_d = nc.dram_tensor("out", [N, C], bf16, kind="ExternalOutput")

    with tile.TileContext(nc) as tc:
        from contextlib import ExitStack

        with ExitStack() as ctx:
            sb = ctx.enter_context(tc.tile_pool(name="sb", bufs=1))
            work = ctx.enter_context(tc.tile_pool(name="work", bufs=4))
            avp = ctx.enter_context(tc.tile_pool(name="avp", bufs=2))
            rcp = ctx.enter_context(tc.tile_pool(name="rcp", bufs=2))
            ps2 = ctx.enter_context(tc.tile_pool(name="ps2", bufs=4, space="PSUM"))
            psav = ctx.enter_context(tc.tile_pool(name="psav", bufs=2, space="PSUM"))
            aux = ctx.enter_context(tc.tile_pool(name="aux", bufs=2, space="PSUM"))

            # ---- persistent SBUF tiles ----
            xt_sb = sb.tile([128, KT, N], bf16, tag="xt")
            wq_sb = sb.tile([128, KTA, FQ], bf16, tag="wq")
            wk_sb = sb.tile([128, KTA, FQ], bf16, tag="wk")
            wv_sb = sb.tile([128, KTA, VEXT], bf16, tag="wv")
            wp_sb = sb.tile([128, 2, C], bf16, tag="wp")
            qT_sb = sb.tile([128, 2, N], bf16, tag="qT")
            kT_sb = sb.tile([128, 2, N], bf16, tag="kT")
            v_sb = sb.tile([128, NT, VEXT], bf16, tag="v")
            ao_sb = sb.tile([128, 2, N], bf16, tag="ao")
            ones_sb = sb.tile([128, 512], bf16, tag="ones")
            ind_sb = sb.tile([HPC, HPC, HD], bf16, tag="ind")
            den_sb = sb.tile([HPC, NQC, 512], bf16, tag="den")

            # ---- input DMAs (spread over 3 queues; ordered so QKV compute
            # can start early) ----
            for t in range(KTA):
                nc.scalar.dma_start(out=wq_sb[:, t, :], in_=wq_d[t])
                nc.scalar.dma_start(out=wk_sb[:, t, :], in_=wk_d[t])
            for c4 in range(4):
                for t in range(KT):
                    nc.sync.dma_start(
                        out=xt_sb[:, t, ts(c4, 512)], in_=xt_d[t][:, ts(c4, 512)]
                    )
            for t in range(KTA):
                nc.gpsimd.dma_start(out=wv_sb[:, t, :], in_=wv_d[t])
            for t in range(2):
                nc.gpsimd.dma_start(out=wp_sb[:, t, :], in_=wp_d[t])

            nc.vector.memset(ones_sb, 0.0)
            nc.vector.memset(ones_sb[0:1, :], 1.0)
            nc.sync.dma_start(out=ind_sb[:, :, :], in_=ind_d[:, :, :])

            # ---- PE warm-up: dummy matmuls while input DMAs land, so the
            # HAM clock gate is released before real work starts ----
            wu_out = sb.tile([128, 512], bf16, tag="wuout")
            wu = ps2.tile([128, 2, 512], f32, tag="sc", bufs=2, name="wu")
            for i in range(8):
                nc.tensor.matmul(
                    wu[:, 0, :],
                    ones_sb[:, 0:128],
                    ones_sb,
                    start=(i == 0),
                    stop=(i == 7),
                    skip_group_check=True,
                )
            nc.vector.tensor_copy(out=wu_out, in_=wu[:, 0, :])

            # ---- QKV phase (2-bank PSUM tiles shared with attention) ----
            # qT/kT: [(h,d) partition, n free]; chunk pairs share a tile so
            # one 1024-wide copy drains both
            for cp in range(2):
                for dst_sb, w_sb in ((qT_sb, wq_sb), (kT_sb, wk_sb)):
                    for tout in range(2):
                        pg = ps2.tile([128, 2, 512], f32, tag="sc", bufs=2)
                        for j in range(2):
                            ch = cp * 2 + j
                            for t in range(KTA):
                                rhs = (
                                    xt_sb[:, t, ts(ch, 512)]
                                    if t < KT
                                    else ones_sb[:, :]
                                )
                                nc.tensor.matmul(
                                    pg[:, j, :],
                                    w_sb[:, t, ts(tout, 128)],
                                    rhs,
                                    start=(t == 0),
                                    stop=(t == KTA - 1),
                                    skip_group_check=True,
                                )
                        nc.scalar.copy(
                            out=dst_sb[:, tout, ds(cp * 1024, 1024)], in_=pg
                        )

            # v: [n partition, (h, d|ones) free]; m pairs share a tile
            for mp in range(NT // 2):
                pg = ps2.tile([128, 2, 512], f32, tag="sc", bufs=2)
                for j in range(2):
                    m = mp * 2 + j
                    for t in range(KTA):
                        lhsT = (
                            xt_sb[:, t, ts(m, 128)]
                            if t < KT
                            else ones_sb[:, 0:128]
                        )
                        nc.tensor.matmul(
                            pg[:, j, 0:VEXT],
                            lhsT,
                            wv_sb[:, t, :],
                            start=(t == 0),
                            stop=(t == KTA - 1),
                            skip_group_check=True,
                        )
                nc.vector.tensor_copy(
                    out=v_sb[:, mp * 2 : mp * 2 + 2, :], in_=pg[:, :, 0:VEXT]
                )

            # ---- attention (head pairs) + deferred norm/proj ----
            def make_recip(qc, r0, nr):
                # 1/den via exp(-ln(den)) on ACT: same table set as the
                # attention exps, and it keeps the DVE queue clean.
                def cb():
                    lden = rcp.tile([HPC, 512], f32, tag="lden", bufs=2)
                    nc.scalar.activation(
                        out=lden[0:nr, :],
                        in_=den_sb[r0 : r0 + nr, qc, :],
                        func=mybir.ActivationFunctionType.Ln,
                    )
                    recb = rcp.tile([HPC, 512], bf16, tag="recb", bufs=2)
                    nc.scalar.activation(
                        out=recb[0:nr, :],
                        in_=lden[0:nr, :],
                        func=mybir.ActivationFunctionType.Exp,
                        scale=-1.0,
                    )
                    return recb, nr

                return cb

            def make_norm(av, recb, nr, hsel, h, qc, mul_eng):
                # ao[h] = av_raw[h] * recip[h] (broadcast via indicator matmul)
                t, r = h // 2, (h % 2) * 64

                def cb():
                    pbc = aux.tile([64, 512], f32, tag="aux")
                    nc.tensor.matmul(
                        pbc,
                        ind_sb[0:nr, hsel, :],
                        recb[0:nr, :],
                        start=True,
                        stop=True,
                    )
                    mul_eng.tensor_mul(
                        ao_sb[r : r + 64, t, ts(qc, 512)],
                        pbc,
                        av[0:64, h % 2, :],
                    )

                return cb

            def make_proj(qcp, mq):
                def cb():
                    row0 = qcp * 512 + mq * 128
                    for cc in range(2):
                        psp = aux.tile([128, 512], f32, tag="aux")
                        for t in range(2):
                            nc.tensor.matmul(
                                psp,
                                ao_sb[:, t, ds(row0, 128)],
                                wp_sb[:, t, ts(cc, 512)],
                                start=(t == 0),
                                stop=(t == 1),
                            )
                        oc = work.tile([128, 512], bf16, tag="outc")
                        if cc == 0:
                            nc.vector.tensor_copy(out=oc, in_=psp)
                        else:
                            nc.scalar.copy(out=oc, in_=psp)
                        nc.sync.dma_start(
                            out=out_d[ds(row0, 128), ts(cc, 512)], in_=oc
                        )

                return cb

            # deferred callbacks, keyed to explicit (qc, t, g) slots so the
            # reciprocal chain has drained before the norm matmuls enter the
            # in-order PE queue
            slotted = {}
            leftover = []
            ectr = [0]  # global exp half-tile counter (5:7 ACT:DVE rotation)

            for qc in range(NQC):
                for t in range(2):
                    pav = [
                        psav.tile([65, 512], f32, tag="av", name=f"pav{qc}_{t}_{hh}")
                        for hh in range(2)
                    ]
                    prev = None
                    for g in range(NT // 2):
                        m0 = g * 2
                        # 2-bank score tile per head per step, one exp each:
                        # h0 -> ACT (exact), h1 -> DVE (Schraudolph). Scores
                        # h0/h1 back-to-back: row groups 0/64 run
                        # concurrently in the PE array.
                        pgs = [
                            ps2.tile(
                                [128, 2, 512],
                                f32,
                                tag="sc",
                                bufs=2,
                                name=f"pg{qc}{t}{g}{hh}",
                            )
                            for hh in range(2)
                        ]
                        for j in range(2):
                            for hh in range(2):
                                r = hh * 64
                                nc.tensor.matmul(
                                    pgs[hh][:, j, :],
                                    kT_sb[r : r + 64, t, ts(m0 + j, 128)],
                                    qT_sb[r : r + 64, t, ts(qc, 512)],
                                    start=True,
                                    stop=True,
                                    skip_group_check=True,
                                )
                        at = [
                            work.tile(
                                [128, 2, 512],
                                bf16,
                                tag="attnT",
                                bufs=4,
                                name=f"at{qc}{t}{g}{hh}",
                            )
                            for hh in range(2)
                        ]
                        nc.scalar.activation(
                            out=at[0],
                            in_=pgs[0],
                            func=mybir.ActivationFunctionType.Exp,
                            scale=1.0 / A_SCH,
                        )
                        nc.vector.tensor_scalar(
                            out=at[1].bitcast(u16),
                            in0=pgs[1],
                            scalar1=B_SCH,
                            scalar2=None,
                            op0=mybir.AluOpType.add,
                        )
                        # one deferred callback per step keeps PE fed
                        cb = slotted.pop((qc, t, g), None)
                        if cb is not None:
                            cb()
                        # AV for previous step
                        if prev is not None:
                            pat, pm0 = prev
                            for hh in range(2):
                                for j in range(2):
                                    nc.tensor.matmul(
                                        pav[hh],
                                        v_sb[
                                            :,
                                            pm0 + j,
                                            ds((2 * t + hh) * 65, 65),
                                        ],
                                        pat[hh][:, j, :],
                                        start=(pm0 + j == 0),
                                        stop=False,
                                        skip_group_check=True,
                                    )
                        prev = (at, m0)
                    # pair tail: last AV group, then drain pav to SBUF
                    pat, pm0 = prev
                    for hh in range(2):
                        for j in range(2):
                            nc.tensor.matmul(
                                pav[hh],
                                v_sb[:, pm0 + j, ds((2 * t + hh) * 65, 65)],
                                pat[hh][:, j, :],
                                start=False,
                                stop=(j == 1),
                                skip_group_check=True,
                            )
                    av = avp.tile([65, 2, 512], bf16, tag="avraw")
                    for hh in range(2):
                        if hh == 0:
                            nc.scalar.copy(
                                out=av[:, hh, :], in_=pav[hh][0:65, :]
                            )
                        else:
                            nc.vector.tensor_copy(
                                out=av[:, hh, :], in_=pav[hh][0:65, :]
                            )
                        nc.sync.dma_start(
                            out=den_sb[2 * t + hh : 2 * t + hh + 1, qc, :],
                            in_=av[64:65, hh, :],
                        )
                    if t == 0:
                        av0 = av
                        if qc == NQC - 1:
                            # last chunk: advance h0/h1 norms into the t=1
                            # steps via a 2-row reciprocal
                            box01 = []

                            def recip01_cb(qc=qc):
                                box01.append(make_recip(qc, 0, 2)())

                            slotted[(qc, 1, 1)] = recip01_cb
                            for h in range(2):

                                def n_cb(h=h, qc=qc, avx=av):
                                    recb, nr = box01[0]
                                    make_norm(
                                        avx, recb, nr, h, h, qc, nc.vector
                                    )()

                                slotted[(qc, 1, 3 + 2 * h)] = n_cb
                    else:
                        # qc complete: slot recip early, norms after the ACT
                        # chain has drained, proj after the norms
                        def make_qc_tail(qc=qc, av0=av0, av1=av):
                            recb_box = []

                            def recip_cb():
                                recb_box.append(make_recip(qc, 0, HPC)())

                            cbs = [((0, 0), recip_cb)]
                            for h in range(HPC):
                                avx = av0 if h < 2 else av1

                                def norm_cb(h=h, avx=avx):
                                    recb, nr = recb_box[0]
                                    make_norm(
                                        avx, recb, nr, h, h, qc, nc.vector
                                    )()

                                cbs.append(((0, 3 + h), norm_cb))
                            for mq in range(4):
                                cbs.append(((1, 2 * mq), make_proj(qc, mq)))
                            return cbs

                        if qc + 1 < NQC:
                            for (tt, gg), cb in make_qc_tail():
                                slotted[(qc + 1, tt, gg)] = cb
                        else:
                            # flush tail: 2-row recip for h2/h3, DVE muls,
                            # then the final projections
                            av0x, av1x = av0, av
                            box23 = []

                            def recip23_cb(qc=qc):
                                box23.append(make_recip(qc, 0, HPC)())

                            leftover.append(recip23_cb)
                            for h in range(2, HPC):

                                def n_cb(h=h, qc=qc, avx=av):
                                    recb, nr = box23[0]
                                    make_norm(
                                        avx, recb, nr, h, h, qc, nc.vector
                                    )()

                                leftover.append(n_cb)
                            for mq in range(4):
                                leftover.append(make_proj(qc, mq))
            for cb in leftover:
                cb()
    if hw_passes:
        _strip_self_waits(nc)
        _split_multi_waits(nc)
    return nc


def _split_multi_waits(nc):
    # core_v2/v3 codegen allows one sync wait per instruction; hoist extra
    # waits onto same-engine nops inserted immediately before (wait point
    # unchanged, so no deadlock risk).
    import bass_rust

    qmap = {
        "Activation": nc.scalar,
        "PE": nc.tensor,
        "DVE": nc.vector,
        "Pool": nc.gpsimd,
        "SP": nc.sync,
    }
    for bbh in list(nc.bb_map.values()):
        lst = bbh.bb.instructions
        idx = 0
        while idx < len(lst):
            ins = lst[idx]
            si = getattr(ins, "sync_info", None)
            if si is not None and si.on_wait and len(si.on_wait) > 1:
                waits = list(si.on_wait)
                eng = str(ins.engine).split(".")[-1]
                q = qmap[eng]
                for w in waits[:-1]:
                    bi = q.nop(hint="xw", nofuse=True)
                    nop_ins = bi.ins if hasattr(bi, "ins") else bi
                    cur_lst = nc.cur_bb.bb.instructions
                    assert cur_lst[-1].name == nop_ins.name
                    cur_lst.pop()
                    nop_ins.sync_info = bass_rust.SyncInfo(
                        on_wait=[w], on_update=[]
                    )
                    lst.insert(idx, nop_ins)
                    idx += 1
                si.on_wait = waits[-1:]
            idx += 1


def _strip_self_waits(nc):
    # optimize_sems is disabled upstream; remove provably-redundant
    # same-queue waits (in-order queues guarantee them) so no instruction
    # exceeds core_v2's per-instruction sync-wait slot limit.
    counts = {}
    for ins in nc.all_instructions():
        si = getattr(ins, "sync_info", None)
        if si is None:
            continue
        ups = [u for u in (si.on_update or []) if u.update_mode == "sem-inc"]
        own = {u.ant_name for u in ups}
        waits = list(si.on_wait or [])
        if waits:
            kept = [
                w
                for w in waits
                if not (
                    w.wait_mode == "sem-ge-imm"
                    and w.ant_name in own
                    and w.wait_value <= counts.get(w.ant_name, 0)
                )
            ]
            if len(kept) != len(waits):
                si.on_wait = kept
        for u in ups:
            counts[u.ant_name] = counts.get(u.ant_name, 0) + u.update_value
    return nc


_NC = None


def _install_ntff_hook():
    """Provide antenv.axon_hooks via ctypes if the image lacks it."""
    import sys as _sys

    try:
        from antenv.axon_hooks import get_axon_ntff_profile_hook  # noqa: F401

        return
    except ImportError:
        pass

    import contextlib
    import ctypes
    import types

    so_path = "/opt/axon/libaxon_pjrt.so"
    hook = None
    if os.path.exists(so_path):
        lib = ctypes.CDLL(so_path)
        if hasattr(lib, "axon_start_nrt_profile"):
            lib.axon_start_nrt_profile.argtypes = [
                ctypes.POINTER(ctypes.c_int64),
                ctypes.c_size_t,
            ]
            lib.axon_start_nrt_profile.restype = ctypes.c_int64
            lib.axon_stop_nrt_profile.argtypes = [ctypes.c_char_p]
            lib.axon_stop_nrt_profile.restype = ctypes.c_int64

            @contextlib.contextmanager
            def hook(output_dir, device_ids):
                import jax

                jax.devices()
                if device_ids:
                    ids = (ctypes.c_int64 * len(device_ids))(*device_ids)
                    rc = lib.axon_start_nrt_profile(ids, len(device_ids))
                else:
                    rc = lib.axon_start_nrt_profile(None, 0)
                if rc != 0:
                    raise RuntimeError(f"axon_start_nrt_profile rc={rc}")
                try:
                    yield
                finally:
                    n = lib.axon_stop_nrt_profile(str(output_dir).encode())
                    if n < 0:
                        raise RuntimeError(f"axon_stop_nrt_profile rc={n}")

    mod = types.ModuleType("antenv.axon_hooks")
    mod.get_axon_ntff_profile_hook = lambda: hook
    try:
        import antenv

        antenv.axon_hooks = mod
    except ImportError:
        pkg = types.ModuleType("antenv")
        pkg.axon_hooks = mod
        pkg.__path__ = []
        _sys.modules["antenv"] = pkg
    _sys.modules["antenv.axon_hooks"] = mod


def _get_nc():
    global _NC
    if _NC is None:
        _NC = _build_nc()
    return _NC


def _prep_inputs(x, W_qkv, b_qkv):
    """Per-core host-side pre-layout (bf16, matmul-ready)."""
    xt = {}
    for b in range(B):
        xt[b] = np.ascontiguousarray(
            x[b].T.reshape(KT, 128, N)
        ).astype(BF16)

    maps = []
    for c in range(NCORES):
        b = c // 4
        hs = (c % 4) * HPC
        col0 = hs * HD

        wq_aug = np.zeros((KTA * 128, FQ), np.float32)
        wq_aug[0:C] = W_qkv[:, col0 : col0 + FQ] * S_QK
        wq_aug[C] = b_qkv[col0 : col0 + FQ] * S_QK

        wk_aug = np.zeros((KTA * 128, FQ), np.float32)
        wk_aug[0:C] = W_qkv[:, C + col0 : C + col0 + FQ] * S_QK
        wk_aug[C] = b_qkv[C + col0 : C + col0 + FQ] * S_QK

        wv_aug = np.zeros((KTA * 128, VEXT), np.float32)
        for h in range(HPC):
            g = 2 * C + (hs + h) * HD
            wv_aug[0:C, h * 65 : h * 65 + HD] = W_qkv[:, g : g + HD]
            wv_aug[C, h * 65 : h * 65 + HD] = b_qkv[g : g + HD]
            wv_aug[C, h * 65 + HD] = 1.0

        maps.append(
            {
                "xt": xt[b],
                "wq": np.ascontiguousarray(wq_aug.reshape(KTA, 128, FQ)).astype(BF16),
                "wk": np.ascontiguousarray(wk_aug.reshape(KTA, 128, FQ)).astype(BF16),
                "wv": np.ascontiguousarray(wv_aug.reshape(KTA, 128, VEXT)).astype(BF16),
                "wp": None,  # filled below
                "ind": _IND,
            }
        )
    return maps


_IND = np.zeros((HPC, HPC, HD), BF16)
for _h in range(HPC):
    _IND[_h, _h, :] = 1.0


def kernel(x, W_qkv, b_qkv, W_proj, b_proj):
    from concourse.bass_utils import run_bass_kernel_spmd

    nc = _get_nc()
    in_maps = _prep_inputs(x, W_qkv, b_qkv)
    for c in range(NCORES):
        hs = (c % 4) * HPC
        r0 = hs * HD
        wp_slice = W_proj[r0 : r0 + FQ, :]
        in_maps[c]["wp"] = np.ascontiguousarray(
            wp_slice.reshape(2, 128, C)
        ).astype(BF16)

    trace = bool(os.environ.get("KERNEL_TRACE"))
    if trace:
        _install_ntff_hook()
    try:
        res = run_bass_kernel_spmd(nc, in_maps, list(range(NCORES)), trace=trace)
    except Exception:
        if not trace:
            raise
        res = run_bass_kernel_spmd(nc, in_maps, list(range(NCORES)), trace=False)
    kernel.last_results = res

    out = np.zeros((B, N, C), np.float32)
    for c in range(NCORES):
        out[c // 4] += res.results[c]["out"].astype(np.float32)
    out += b_proj.astype(np.float32)
    return out


# revision 59
# speedup vs baseline: 1.1365x; 1.1365x over previous
import os
import sys

sys.path.insert(0, "/opt/trn_rl_repo")

import numpy as np
import ml_dtypes

import concourse.bass as bass
import concourse.bacc as bacc
import concourse.tile as tile
from concourse import mybir
from concourse.bass import ds, ts

BF16 = ml_dtypes.bfloat16

B, N, C = 2, 2048, 1024
H = 16
HD = C // H          # 64
HPC = 4              # heads per core
NCORES = 8
SCALE = HD ** -0.5   # 0.125
KT = C // 128        # 8 k-tiles over the C contraction
KTA = KT + 1         # +1 aug tile (bias / ones row)
FQ = HPC * HD        # 256 per-core q/k feature dim
VEXT = HPC * (HD + 1)  # 260: per-head [64 v cols | 1 ones col]
NT = N // 128        # 16 row tiles
NQC = N // 512       # 4 query chunks of 512

# Schraudolph bf16 exp: bits(exp(s)) ~= round(A_SCH*s + B_SCH), read as bf16.
# q,k weights are pre-scaled by sqrt(SCALE*A_SCH) so the score PSUM arrives
# already in the Schraudolph domain; the exact-exp path divides back out.
A_SCH = 184.6650292  # 2^7 / ln 2
C_SCH = 7.5          # bias correction (tuned for softmax-normalized output)
B_SCH = 16256.0 - C_SCH
S_QK = float(np.sqrt(SCALE * A_SCH))


def _build_nc(hw_passes: bool = True) -> bass.Bass:
    nc = bass.Bass()
    f32 = mybir.dt.float32
    bf16 = mybir.dt.bfloat16
    u16 = mybir.dt.uint16

    xt_d = nc.dram_tensor("xt", [KT, 128, N], bf16, kind="ExternalInput")
    wq_d = nc.dram_tensor("wq", [KTA, 128, FQ], bf16, kind="ExternalInput")
    wk_d = nc.dram_tensor("wk", [KTA, 128, FQ], bf16, kind="ExternalInput")
    wv_d = nc.dram_tensor("wv", [KTA, 128, VEXT], bf16, kind="ExternalInput")
    wp_d = nc.dram_tensor("wp", [2, 128, C], bf16, kind="ExternalInput")
    ind_d = nc.dram_tensor("ind", [HPC, HPC, HD], bf16, kind="ExternalInput")
    out_d = nc.dram_tensor("out", [N, C], bf16, kind="ExternalOutput")

    with tile.TileContext(nc) as tc:
        from contextlib import ExitStack

        with ExitStack() as ctx:
            sb = ctx.enter_context(tc.tile_pool(name="sb", bufs=1))
            work = ctx.enter_context(tc.tile_pool(name="work", bufs=4))
            avp = ctx.enter_context(tc.tile_pool(name="avp", bufs=2))
            rcp = ctx.enter_context(tc.tile_pool(name="rcp", bufs=2))
            ps2 = ctx.enter_context(tc.tile_pool(name="ps2", bufs=4, space="PSUM"))
            psav = ctx.enter_context(tc.tile_pool(name="psav", bufs=2, space="PSUM"))
            aux = ctx.enter_context(tc.tile_pool(name="aux", bufs=2, space="PSUM"))

            # ---- persistent SBUF tiles ----
            xt_sb = sb.tile([128, KT, N], bf16, tag="xt")
            wq_sb = sb.tile([128, KTA, FQ], bf16, tag="wq")
            wk_sb = sb.tile([128, KTA, FQ], bf16, tag="wk")
            wv_sb = sb.tile([128, KTA, VEXT], bf16, tag="wv")
            wp_sb = sb.tile([128, 2, C], bf16, tag="wp")
            qT_sb = sb.tile([128, 2, N], bf16, tag="qT")
            kT_sb = sb.tile([128, 2, N], bf16, tag="kT")
            v_sb = sb.tile([128, NT, VEXT], bf16, tag="v")
            ao_sb = sb.tile([128, 2, N], bf16, tag="ao")
            ones_sb = sb.tile([128, 512], bf16, tag="ones")
            ind_sb = sb.tile([HPC, HPC, HD], bf16, tag="ind")
            den_sb = sb.tile([HPC, NQC, 512], bf16, tag="den")

            # ---- input DMAs (spread over 3 queues; ordered so QKV compute
            # can start early) ----
            for t in range(KTA):
                nc.scalar.dma_start(out=wq_sb[:, t, :], in_=wq_d[t])
                nc.scalar.dma_start(out=wk_sb[:, t, :], in_=wk_d[t])
            for c4 in range(4):
                for t in range(KT):
                    nc.sync.dma_start(
                        out=xt_sb[:, t, ts(c4, 512)], in_=xt_d[t][:, ts(c4, 512)]
                    )
            for t in range(KTA):
                nc.gpsimd.dma_start(out=wv_sb[:, t, :], in_=wv_d[t])
            for t in range(2):
                nc.gpsimd.dma_start(out=wp_sb[:, t, :], in_=wp_d[t])

            nc.vector.memset(ones_sb, 0.0)
            nc.vector.memset(ones_sb[0:1, :], 1.0)
            nc.sync.dma_start(out=ind_sb[:, :, :], in_=ind_d[:, :, :])

            # ---- PE warm-up: dummy matmuls while input DMAs land, so the
            # HAM clock gate is released before real work starts ----
            wu_out = sb.tile([128, 512], bf16, tag="wuout")
            wu = ps2.tile([128, 512], f32, tag="sc", name="wu")
            for i in range(8):
                nc.tensor.matmul(
                    wu,
                    ones_sb[:, 0:128],
                    ones_sb,
                    start=(i == 0),
                    stop=(i == 7),
                    skip_group_check=True,
                )
            nc.vector.tensor_copy(out=wu_out, in_=wu)

            # ---- QKV phase (1-bank PSUM tiles shared with attention) ----
            # qT/kT: [(h,d) partition, n free]
            for cp in range(2):
                for j in range(2):
                    for ti, (dst_sb, w_sb) in enumerate(
                        ((qT_sb, wq_sb), (kT_sb, wk_sb))
                    ):
                        for tout in range(2):
                            ch = cp * 2 + j
                            pg = ps2.tile(
                                [128, 512],
                                f32,
                                tag="sc",
                                name=f"qk{cp}{ti}{tout}{j}",
                            )
                            for t in range(KTA):
                                rhs = (
                                    xt_sb[:, t, ts(ch, 512)]
                                    if t < KT
                                    else ones_sb[:, :]
                                )
                                nc.tensor.matmul(
                                    pg,
                                    w_sb[:, t, ts(tout, 128)],
                                    rhs,
                                    start=(t == 0),
                                    stop=(t == KTA - 1),
                                    skip_group_check=True,
                                )
                            drain = nc.scalar.copy if j == 0 else nc.vector.tensor_copy
                            drain(
                                out=dst_sb[:, tout, ts(ch, 512)], in_=pg
                            )

            # v: [n partition, (h, d|ones) free]
            for m in range(NT):
                pg = ps2.tile([128, 512], f32, tag="sc", name=f"v{m}")
                for t in range(KTA):
                    lhsT = (
                        xt_sb[:, t, ts(m, 128)]
                        if t < KT
                        else ones_sb[:, 0:128]
                    )
                    nc.tensor.matmul(
                        pg[:, 0:VEXT],
                        lhsT,
                        wv_sb[:, t, :],
                        start=(t == 0),
                        stop=(t == KTA - 1),
                        skip_group_check=True,
                    )
                drain = nc.vector.tensor_copy if m % 2 == 0 else nc.scalar.copy
                drain(out=v_sb[:, m, :], in_=pg[:, 0:VEXT])

            # ---- attention (head pairs) + deferred norm/proj ----
            def make_recip(qc, r0, nr):
                # 1/den via exp(-ln(den)) on ACT: same table set as the
                # attention exps, and it keeps the DVE queue clean.
                def cb():
                    lden = rcp.tile([HPC, 512], f32, tag="lden", bufs=2)
                    nc.scalar.activation(
                        out=lden[0:nr, :],
                        in_=den_sb[r0 : r0 + nr, qc, :],
                        func=mybir.ActivationFunctionType.Ln,
                    )
                    recb = rcp.tile([HPC, 512], bf16, tag="recb", bufs=2)
                    nc.scalar.activation(
                        out=recb[0:nr, :],
                        in_=lden[0:nr, :],
                        func=mybir.ActivationFunctionType.Exp,
                        scale=-1.0,
                    )
                    return recb, nr

                return cb

            def make_norm(av, recb, nr, hsel, h, qc, mul_eng):
                # ao[h] = av_raw[h] * recip[h] (broadcast via indicator matmul)
                t, r = h // 2, (h % 2) * 64

                def cb():
                    pbc = aux.tile([64, 512], f32, tag="aux")
                    nc.tensor.matmul(
                        pbc,
                        ind_sb[0:nr, hsel, :],
                        recb[0:nr, :],
                        start=True,
                        stop=True,
                    )
                    mul_eng.tensor_mul(
                        ao_sb[r : r + 64, t, ts(qc, 512)],
                        pbc,
                        av[0:64, h % 2, :],
                    )

                return cb

            def make_proj(qcp, mq):
                def cb():
                    row0 = qcp * 512 + mq * 128
                    for cc in range(2):
                        psp = aux.tile([128, 512], f32, tag="aux")
                        for t in range(2):
                            nc.tensor.matmul(
                                psp,
                                ao_sb[:, t, ds(row0, 128)],
                                wp_sb[:, t, ts(cc, 512)],
                                start=(t == 0),
                                stop=(t == 1),
                            )
                        oc = work.tile([128, 512], bf16, tag="outc")
                        if cc == 0:
                            nc.vector.tensor_copy(out=oc, in_=psp)
                        else:
                            nc.scalar.copy(out=oc, in_=psp)
                        nc.sync.dma_start(
                            out=out_d[ds(row0, 128), ts(cc, 512)], in_=oc
                        )

                return cb

            # deferred callbacks, keyed to explicit (qc, t, g) slots so the
            # reciprocal chain has drained before the norm matmuls enter the
            # in-order PE queue
            slotted = {}
            leftover = []
            ectr = [0]  # global exp half-tile counter (5:7 ACT:DVE rotation)

            for qc in range(NQC):
                for t in range(2):
                    pav = [
                        psav.tile([65, 512], f32, tag="av", name=f"pav{qc}_{t}_{hh}")
                        for hh in range(2)
                    ]
                    prev = None
                    for g in range(NT // 2):
                        m0 = g * 2
                        # 4 one-bank score tiles per step; per-bank exps keep
                        # the PSUM ring turning without an exp-latency stall.
                        # scores h0/h1 back-to-back: row groups 0/64 run
                        # concurrently in the PE array.
                        at = {}
                        for j in range(2):
                            pgs = {}
                            for hh in range(2):
                                r = hh * 64
                                pg = ps2.tile(
                                    [128, 512],
                                    f32,
                                    tag="sc",
                                    name=f"pg{qc}{t}{g}{hh}{j}",
                                )
                                nc.tensor.matmul(
                                    pg,
                                    kT_sb[r : r + 64, t, ts(m0 + j, 128)],
                                    qT_sb[r : r + 64, t, ts(qc, 512)],
                                    start=True,
                                    stop=True,
                                    skip_group_check=True,
                                )
                                pgs[hh] = pg
                            for hh in range(2):
                                a = work.tile(
                                    [128, 512],
                                    bf16,
                                    tag="attnT",
                                    bufs=8,
                                    name=f"at{qc}{t}{g}{hh}{j}",
                                )
                                i = ectr[0]
                                ectr[0] += 1
                                if (i * 17) % 32 < 17:
                                    nc.scalar.activation(
                                        out=a,
                                        in_=pgs[hh],
                                        func=mybir.ActivationFunctionType.Exp,
                                        scale=1.0 / A_SCH,
                                    )
                                else:
                                    nc.vector.tensor_scalar(
                                        out=a.bitcast(u16),
                                        in0=pgs[hh],
                                        scalar1=B_SCH,
                                        scalar2=None,
                                        op0=mybir.AluOpType.add,
                                    )
                                at[(hh, j)] = a
                        # one deferred callback per step keeps PE fed
                        cb = slotted.pop((qc, t, g), None)
                        if cb is not None:
                            cb()
                        # AV for previous step
                        if prev is not None:
                            pat, pm0 = prev
                            for hh in range(2):
                                for j in range(2):
                                    nc.tensor.matmul(
                                        pav[hh],
                                        v_sb[
                                            :,
                                            pm0 + j,
                                            ds((2 * t + hh) * 65, 65),
                                        ],
                                        pat[(hh, j)],
                                        start=(pm0 + j == 0),
                                        stop=False,
                                        skip_group_check=True,
                                    )
                        prev = (at, m0)
                    # pair tail: last AV group, then drain pav to SBUF
                    pat, pm0 = prev
                    for hh in range(2):
                        for j in range(2):
                            nc.tensor.matmul(
                                pav[hh],
                                v_sb[:, pm0 + j, ds((2 * t + hh) * 65, 65)],
                                pat[(hh, j)],
                                start=False,
                                stop=(j == 1),
                                skip_group_check=True,
                            )
                    av = avp.tile([65, 2, 512], bf16, tag="avraw")
                    for hh in range(2):
                        if hh == 0:
                            nc.scalar.copy(
                                out=av[:, hh, :], in_=pav[hh][0:65, :]
                            )
                        else:
                            nc.vector.tensor_copy(
                                out=av[:, hh, :], in_=pav[hh][0:65, :]
                            )
                        nc.sync.dma_start(
                            out=den_sb[2 * t + hh : 2 * t + hh + 1, qc, :],
                            in_=av[64:65, hh, :],
                        )
                    if t == 0:
                        av0 = av
                        if qc == NQC - 1:
                            # last chunk: advance h0/h1 norms into the t=1
                            # steps via a 2-row reciprocal
                            box01 = []

                            def recip01_cb(qc=qc):
                                box01.append(make_recip(qc, 0, 2)())

                            slotted[(qc, 1, 1)] = recip01_cb
                            for h in range(2):

                                def n_cb(h=h, qc=qc, avx=av):
                                    recb, nr = box01[0]
                                    make_norm(
                                        avx, recb, nr, h, h, qc, nc.vector
                                    )()

                                slotted[(qc, 1, 3 + 2 * h)] = n_cb
                    else:
                        # qc complete: slot recip early, norms after the ACT
                        # chain has drained, proj after the norms
                        def make_qc_tail(qc=qc, av0=av0, av1=av):
                            recb_box = []

                            def recip_cb():
                                recb_box.append(make_recip(qc, 0, HPC)())

                            cbs = [((0, 0), recip_cb)]
                            for h in range(HPC):
                                avx = av0 if h < 2 else av1

                                def norm_cb(h=h, avx=avx):
                                    recb, nr = recb_box[0]
                                    make_norm(
                                        avx, recb, nr, h, h, qc, nc.vector
                                    )()

                                cbs.append(((0, 3 + h), norm_cb))
                            for mq in range(4):
                                cbs.append(((1, 2 * mq), make_proj(qc, mq)))
                            return cbs

                        if qc + 1 < NQC:
                            for (tt, gg), cb in make_qc_tail():
                                slotted[(qc + 1, tt, gg)] = cb
                        else:
                            # flush tail: 2-row recip for h2/h3, DVE muls,
                            # then the final projections
                            av0x, av1x = av0, av
                            box23 = []

                            def recip23_cb(qc=qc):
                                box23.append(make_recip(qc, 0, HPC)())

                            leftover.append(recip23_cb)
                            for h in range(2, HPC):

                                def n_cb(h=h, qc=qc, avx=av):
                                    recb, nr = box23[0]
                                    make_norm(
                                        avx, recb, nr, h, h, qc, nc.vector
                                    )()

                                leftover.append(n_cb)
                            for mq in range(4):
                                leftover.append(make_proj(qc, mq))
            for cb in leftover:
                cb()
    if hw_passes:
        _strip_self_waits(nc)
        _split_multi_waits(nc)
    return nc


def _split_multi_waits(nc):
    # core_v2/v3 codegen allows one sync wait per instruction; hoist extra
    # waits onto same-engine nops inserted immediately before (wait point
    # unchanged, so no deadlock risk).
    import bass_rust

    qmap = {
        "Activation": nc.scalar,
        "PE": nc.tensor,
        "DVE": nc.vector,
        "Pool": nc.gpsimd,
        "SP": nc.sync,
    }
    for bbh in list(nc.bb_map.values()):
        lst = bbh.bb.instructions
        idx = 0
        while idx < len(lst):
            ins = lst[idx]
            si = getattr(ins, "sync_info", None)
            if si is not None and si.on_wait and len(si.on_wait) > 1:
                waits = list(si.on_wait)
                eng = str(ins.engine).split(".")[-1]
                q = qmap[eng]
                for w in waits[:-1]:
                    bi = q.nop(hint="xw", nofuse=True)
                    nop_ins = bi.ins if hasattr(bi, "ins") else bi
                    cur_lst = nc.cur_bb.bb.instructions
                    assert cur_lst[-1].name == nop_ins.name
                    cur_lst.pop()
                    nop_ins.sync_info = bass_rust.SyncInfo(
                        on_wait=[w], on_update=[]
                    )
                    lst.insert(idx, nop_ins)
                    idx += 1
                si.on_wait = waits[-1:]
            idx += 1


def _strip_self_waits(nc):
    # optimize_sems is disabled upstream; remove provably-redundant
    # same-queue waits (in-order queues guarantee them) so no instruction
    # exceeds core_v2's per-instruction sync-wait slot limit.
    counts = {}
    for ins in nc.all_instructions():
        si = getattr(ins, "sync_info", None)
        if si is None:
            continue
        ups = [u for u in (si.on_update or []) if u.update_mode == "sem-inc"]
        own = {u.ant_name for u in ups}
        waits = list(si.on_wait or [])
        if waits:
            kept = [
                w
                for w in waits
                if not (
                    w.wait_mode == "sem-ge-imm"
                    and w.ant_name in own
                    and w.wait_value <= counts.get(w.ant_name, 0)
                )
            ]
            if len(kept) != len(waits):
                si.on_wait = kept
        for u in ups:
            counts[u.ant_name] = counts.get(u.ant_name, 0) + u.update_value
    return nc


_NC = None


def _install_ntff_hook():
    """Provide antenv.axon_hooks via ctypes if the image lacks it."""
    import sys as _sys

    try:
        from antenv.axon_hooks import get_axon_ntff_profile_hook  # noqa: F401

        return
    except ImportError:
        pass

    import contextlib
    import ctypes
    import types

    so_path = "/opt/axon/libaxon_pjrt.so"
    hook = None
    if os.path.exists(so_path):
        lib = ctypes.CDLL(so_path)
        if hasattr(lib, "axon_start_nrt_profile"):
            lib.axon_start_nrt_profile.argtypes = [
                ctypes.POINTER(ctypes.c_int64),
                ctypes.c_size_t,
            ]
            lib.axon_start_nrt_profile.restype = ctypes.c_int64
            lib.axon_stop_nrt_profile.argtypes = [ctypes.c_char_p]
            lib.axon_stop_nrt_profile.restype = ctypes.c_int64

            @contextlib.contextmanager
            def hook(output_dir, device_ids):
                import jax

                jax.devices()
                if device_ids:
                    ids = (ctypes.c_int64 * len(device_ids))(*device_ids)
                    rc = lib.axon_start_nrt_profile(ids, len(device_ids))
                else:
                    rc = lib.axon_start_nrt_profile(None, 0)
                if rc != 0:
                    raise RuntimeError(f"axon_start_nrt_profile rc={rc}")
                try:
                    yield
                finally:
                    n = lib.axon_stop_nrt_profile(str(output_dir).encode())
                    if n < 0:
                        raise RuntimeError(f"axon_stop_nrt_profile rc={n}")

    mod = types.ModuleType("antenv.axon_hooks")
    mod.get_axon_ntff_profile_hook = lambda: hook
    try:
        import antenv

        antenv.axon_hooks = mod
    except ImportError:
        pkg = types.ModuleType("antenv")
        pkg.axon_hooks = mod
        pkg.__path__ = []
        _sys.modules["antenv"] = pkg
    _sys.modules["antenv.axon_hooks"] = mod


def _get_nc():
    global _NC
    if _NC is None:
        _NC = _build_nc()
    return _NC


def _prep_inputs(x, W_qkv, b_qkv):
    """Per-core host-side pre-layout (bf16, matmul-ready)."""
    xt = {}
    for b in range(B):
        xt[b] = np.ascontiguousarray(
            x[b].T.reshape(KT, 128, N)
        ).astype(BF16)

    maps = []
    for c in range(NCORES):
        b = c // 4
        hs = (c % 4) * HPC
        col0 = hs * HD

        wq_aug = np.zeros((KTA * 128, FQ), np.float32)
        wq_aug[0:C] = W_qkv[:, col0 : col0 + FQ] * S_QK
        wq_aug[C] = b_qkv[col0 : col0 + FQ] * S_QK

        wk_aug = np.zeros((KTA * 128, FQ), np.float32)
        wk_aug[0:C] = W_qkv[:, C + col0 : C + col0 + FQ] * S_QK
        wk_aug[C] = b_qkv[C + col0 : C + col0 + FQ] * S_QK

        wv_aug = np.zeros((KTA * 128, VEXT), np.float32)
        for h in range(HPC):
            g = 2 * C + (hs + h) * HD
            wv_aug[0:C, h * 65 : h * 65 + HD] = W_qkv[:, g : g + HD]
            wv_aug[C, h * 65 : h * 65 + HD] = b_qkv[g : g + HD]
            wv_aug[C, h * 65 + HD] = 1.0

        maps.append(
            {
                "xt": xt[b],
                "wq": np.ascontiguousarray(wq_aug.reshape(KTA, 128, FQ)).astype(BF16),
                "wk": np.ascontiguousarray(wk_aug.reshape(KTA, 128, FQ)).astype(BF16),
                "wv": np.ascontiguousarray(wv_aug.reshape(KTA, 128, VEXT)).astype(BF16),
                "wp": None,  # filled below
                "ind": _IND,
            }
        )
    return maps


_IND = np.zeros((HPC, HPC, HD), BF16)
for _h in range(HPC):
    _IND[_h, _h, :] = 1.0


def kernel(x, W_qkv, b_qkv, W_proj, b_proj):
    from concourse.bass_utils import run_bass_kernel_spmd

    nc = _get_nc()
    in_maps = _prep_inputs(x, W_qkv, b_qkv)
    for c in range(NCORES):
        hs = (c % 4) * HPC
        r0 = hs * HD
        wp_slice = W_proj[r0 : r0 + FQ, :]
        in_maps[c]["wp"] = np.ascontiguousarray(
            wp_slice.reshape(2, 128, C)
        ).astype(BF16)

    trace = bool(os.environ.get("KERNEL_TRACE"))
    if trace:
        _install_ntff_hook()
    try:
        res = run_bass_kernel_spmd(nc, in_maps, list(range(NCORES)), trace=trace)
    except Exception:
        if not trace:
            raise
        res = run_bass_kernel_spmd(nc, in_maps, list(range(NCORES)), trace=False)
    kernel.last_results = res

    out = np.zeros((B, N, C), np.float32)
    for c in range(NCORES):
        out[c // 4] += res.results[c]["out"].astype(np.float32)
    out += b_proj.astype(np.float32)
    return out
